# revision 5
# baseline (speedup 1.0000x reference)
"""Causal 7-tap sliding-window kernel for Trainium2 (Bass, pure DMA).

out[b, i, j, c] = x[b, i - (6 - j), c], zeros where the index is negative.

Key fact: out[b, i, :, :] (7*128 floats = 3584 B) is the contiguous slice
x[b, i-6 : i+1, :].  So the whole output is produced by DMA alone:

 - shard batch 32 -> 8 cores x 4 batches (pure data parallel)
 - per core, stage x in SBUF as overlapping chunks: chunk (b, c) holds
   x[b, 125c-6 : 125c+125, :] (131 rows, 67072 B) on partition 32b + 2c,
   so every output window lives inside a single partition
 - chunk (b, 0) instead holds [zeros(6 rows); x[b, 0:125]] (zero halo)
 - store: per batch, a 3-dim DMA whose source slides a 3584 B window in
   512 B steps across the chunks and whose destination is one contiguous
   7.17 MB stream of out[b]

HBM traffic per core: 4.3 MB read + 28.7 MB write  (roofline ~92 us).
"""
import sys

import numpy as np

sys.path.insert(0, "/opt/trn_rl_repo")

import concourse.bass as bass
import concourse.mybir as mybir
from concourse.bass_utils import run_bass_kernel_spmd

DT = mybir.dt.float32
N_CORES = 8
B_FULL = 32    # full batch
B = 4          # batches per core
S = 2000       # sequence length
C = 128        # channels
TAPS = 7
NCHUNK = 16    # chunks per batch
ROWS = S // NCHUNK          # 125 output rows per chunk
CROWS = ROWS + TAPS - 1     # 131 rows stored per chunk
RowE = C                    # 128 elems per row (512 B)
CHUNK_E = CROWS * RowE      # 16768 elems per chunk (67072 B)
WIN_E = TAPS * RowE         # 896 elems per output row (3584 B)
HALO_E = (TAPS - 1) * RowE  # 768 elems of zero halo


def build(sim_pad: bool = False, detect_races: bool = True) -> bass.Bass:
    nc = bass.Bass(detect_race_conditions=detect_races)
    x = nc.declare_dram_parameter("x", [B, S, C], DT, isOutput=False)
    out = nc.declare_dram_parameter("out", [B, S, TAPS, C], DT, isOutput=True)

    # chunk (b, c) -> partition 16b + c: all 16 SBUF-port parities used per
    # batch (even-only layouts halve SBUF read BW), batch extents disjoint.
    def emit_stores(eng, b, store_sem):
        # out[b] is one contiguous 7.17 MB stream; window i reads
        # 3584 B at byte offset 512*(i%125) of partition 16b + i//125.
        # Chunk 0 split from chunks 1-15 (CoreSim init-tracker quirk).
        src_a = bass.AP(
            xs,
            16 * b * CHUNK_E,
            [[CHUNK_E, 1], [RowE, ROWS], [1, WIN_E]],
        )
        dst_a = bass.AP(out, b * S * WIN_E, [[1, ROWS * WIN_E]])
        eng.dma_start(out=dst_a, in_=src_a).then_inc(store_sem, 16)
        src_b = bass.AP(
            xs,
            (16 * b + 1) * CHUNK_E,
            [[CHUNK_E, NCHUNK - 1], [RowE, ROWS], [1, WIN_E]],
        )
        dst_b = bass.AP(
            out,
            (b * S + ROWS) * WIN_E,
            [[1, (S - ROWS) * WIN_E]],
        )
        eng.dma_start(out=dst_b, in_=src_b).then_inc(store_sem, 16)

    with (
        nc.sbuf_tensor([128, CHUNK_E], DT) as xs,
        nc.semaphore("z_sem") as z_sem,
        nc.semaphore("ld0") as ld0,
        nc.semaphore("ld1") as ld1,
        nc.semaphore("ld2") as ld2,
        nc.semaphore("ld3") as ld3,
        nc.semaphore("st_sp") as st_sp,
        nc.semaphore("st_act") as st_act,
        nc.Block() as block,
    ):
        ld = [ld0, ld1, ld2, ld3]

        @block.vector
        def _(vector):
            if sim_pad:
                # CoreSim's uninitialized-memory tracker false-positives on
                # the window reads; pre-touch everything (sim builds only).
                vector.memset(xs[:, :], 0.0)
            # zero halo on every partition (DVE memsets must start at a
            # 32-aligned partition; loads overwrite it on non-halo chunks)
            vector.memset(xs[:, 0:HALO_E], 0.0).then_inc(z_sem, 1)

        @block.gpsimd
        def _(gpsimd):
            gpsimd.wait_ge(z_sem, 1)  # loads overwrite the halo memset
            for b in range(B):
                p0 = 16 * b
                # c=0: x[b, 0:125, :] -> partition 16b, after the halo
                dst0 = bass.AP(
                    xs, p0 * CHUNK_E + HALO_E, [[CHUNK_E, 1], [1, ROWS * RowE]]
                )
                gpsimd.dma_start(out=dst0, in_=x[b, 0:ROWS, :]).then_inc(ld[b], 16)
                # c=1..15: x[b, 125c-6 : 125c+125, :] -> partition 16b+c
                src = bass.AP(
                    x,
                    b * S * C + (ROWS - (TAPS - 1)) * C,
                    [[ROWS * C, NCHUNK - 1], [1, CHUNK_E]],
                )
                dst = bass.AP(
                    xs, (p0 + 1) * CHUNK_E, [[CHUNK_E, NCHUNK - 1], [1, CHUNK_E]]
                )
                gpsimd.dma_start(out=dst, in_=src).then_inc(ld[b], 16)

        # stores split across both HWDGE rings (two descriptor generators)
        @block.sync
        def _(sync):
            sync.wait_ge(z_sem, 1)
            for b in (0, 1):
                sync.wait_ge(ld[b], 32)
                emit_stores(sync, b, st_sp)
            sync.wait_ge(st_sp, 64)
            sync.wait_ge(st_act, 64)

        @block.scalar
        def _(scalar):
            scalar.wait_ge(z_sem, 1)
            for b in (2, 3):
                scalar.wait_ge(ld[b], 32)
                emit_stores(scalar, b, st_act)
            scalar.wait_ge(st_act, 64)

    return nc


_nc_cache = None


def _get_nc():
    global _nc_cache
    if _nc_cache is None:
        _nc_cache = build()
    return _nc_cache


def run(x: np.ndarray, trace: bool = False, tmpdir: str | None = None):
    """Run on 8 NeuronCores; returns (out, BassKernelResults)."""
    x = np.ascontiguousarray(np.asarray(x, dtype=np.float32))
    assert x.shape == (B_FULL, S, C), x.shape
    nc = _get_nc()
    in_maps = [
        {"x": x[i * B : (i + 1) * B]} for i in range(N_CORES)
    ]
    res = run_bass_kernel_spmd(
        nc, in_maps, list(range(N_CORES)), trace=trace, tmpdir=tmpdir
    )
    out = np.concatenate([r["out"] for r in res.results], axis=0)
    return out, res


def kernel(x: np.ndarray) -> np.ndarray:
    out, _ = run(x)
    return out


# revision 6
# speedup vs baseline: 1.0087x; 1.0087x over previous
"""Causal 7-tap sliding-window kernel for Trainium2 (Bass, pure DMA).

out[b, i, j, c] = x[b, i - (6 - j), c], zeros where the index is negative.

Key fact: out[b, i, :, :] (7*128 floats = 3584 B) is the contiguous slice
x[b, i-6 : i+1, :].  So the whole output is produced by DMA alone:

 - shard batch 32 -> 8 cores x 4 batches (pure data parallel)
 - per core, stage x in SBUF as overlapping chunks: chunk (b, c) holds
   x[b, 125c-6 : 125c+125, :] (131 rows, 67072 B) on partition 32b + 2c,
   so every output window lives inside a single partition
 - chunk (b, 0) instead holds [zeros(6 rows); x[b, 0:125]] (zero halo)
 - store: per batch, a 3-dim DMA whose source slides a 3584 B window in
   512 B steps across the chunks and whose destination is one contiguous
   7.17 MB stream of out[b]

HBM traffic per core: 4.3 MB read + 28.7 MB write  (roofline ~92 us).
"""
import sys

import numpy as np

sys.path.insert(0, "/opt/trn_rl_repo")

import concourse.bass as bass
import concourse.mybir as mybir
from concourse.bass_utils import run_bass_kernel_spmd

DT = mybir.dt.float32
N_CORES = 8
B_FULL = 32    # full batch
B = 4          # batches per core
S = 2000       # sequence length
C = 128        # channels
TAPS = 7
NCHUNK = 16    # chunks per batch
ROWS = S // NCHUNK          # 125 output rows per chunk
CROWS = ROWS + TAPS - 1     # 131 rows stored per chunk
RowE = C                    # 128 elems per row (512 B)
CHUNK_E = CROWS * RowE      # 16768 elems per chunk (67072 B)
WIN_E = TAPS * RowE         # 896 elems per output row (3584 B)
HALO_E = (TAPS - 1) * RowE  # 768 elems of zero halo


def build(sim_pad: bool = False, detect_races: bool = True) -> bass.Bass:
    nc = bass.Bass(detect_race_conditions=detect_races)
    x = nc.declare_dram_parameter("x", [B, S, C], DT, isOutput=False)
    out = nc.declare_dram_parameter("out", [B, S, TAPS, C], DT, isOutput=True)

    # chunk (b, c) -> partition 16b + c: all 16 SBUF-port parities used per
    # batch (even-only layouts halve SBUF read BW), batch extents disjoint.
    def emit_stores(eng, b, store_sem):
        # out[b] is one contiguous 7.17 MB stream; window i reads
        # 3584 B at byte offset 512*(i%125) of partition 16b + i//125.
        # Chunk 0 split from chunks 1-15 (CoreSim init-tracker quirk).
        src_a = bass.AP(
            xs,
            16 * b * CHUNK_E,
            [[CHUNK_E, 1], [RowE, ROWS], [1, WIN_E]],
        )
        dst_a = bass.AP(out, b * S * WIN_E, [[1, ROWS * WIN_E]])
        eng.dma_start(out=dst_a, in_=src_a).then_inc(store_sem, 16)
        src_b = bass.AP(
            xs,
            (16 * b + 1) * CHUNK_E,
            [[CHUNK_E, NCHUNK - 1], [RowE, ROWS], [1, WIN_E]],
        )
        dst_b = bass.AP(
            out,
            (b * S + ROWS) * WIN_E,
            [[1, (S - ROWS) * WIN_E]],
        )
        eng.dma_start(out=dst_b, in_=src_b).then_inc(store_sem, 16)

    with (
        nc.sbuf_tensor([128, CHUNK_E], DT) as xs,
        nc.semaphore("z_sem") as z_sem,
        nc.semaphore("ld0") as ld0,
        nc.semaphore("ld1") as ld1,
        nc.semaphore("ld2") as ld2,
        nc.semaphore("ld3") as ld3,
        nc.semaphore("st_sp") as st_sp,
        nc.semaphore("st_act") as st_act,
        nc.Block() as block,
    ):
        ld = [ld0, ld1, ld2, ld3]

        @block.vector
        def _(vector):
            if sim_pad:
                # CoreSim's uninitialized-memory tracker false-positives on
                # the window reads; pre-touch everything (sim builds only).
                vector.memset(xs[:, :], 0.0)
            # zero halo on every partition (DVE memsets must start at a
            # 32-aligned partition; loads overwrite it on non-halo chunks)
            vector.memset(xs[:, 0:HALO_E], 0.0).then_inc(z_sem, 1)

        @block.scalar
        def _(scalar):
            scalar.wait_ge(z_sem, 1)  # loads overwrite the halo memset
            for b in range(B):
                p0 = 16 * b
                # c=0: x[b, 0:125, :] -> partition 16b, after the halo
                dst0 = bass.AP(
                    xs, p0 * CHUNK_E + HALO_E, [[CHUNK_E, 1], [1, ROWS * RowE]]
                )
                scalar.dma_start(out=dst0, in_=x[b, 0:ROWS, :]).then_inc(ld[b], 16)
                # c=1..15: x[b, 125c-6 : 125c+125, :] -> partition 16b+c
                src = bass.AP(
                    x,
                    b * S * C + (ROWS - (TAPS - 1)) * C,
                    [[ROWS * C, NCHUNK - 1], [1, CHUNK_E]],
                )
                dst = bass.AP(
                    xs, (p0 + 1) * CHUNK_E, [[CHUNK_E, NCHUNK - 1], [1, CHUNK_E]]
                )
                scalar.dma_start(out=dst, in_=src).then_inc(ld[b], 16)

        @block.sync
        def _(sync):
            sync.wait_ge(z_sem, 1)
            for b in range(B):
                sync.wait_ge(ld[b], 32)
                emit_stores(sync, b, st_sp)
            sync.wait_ge(st_sp, 128)

    return nc


_nc_cache = None


def _get_nc():
    global _nc_cache
    if _nc_cache is None:
        _nc_cache = build()
    return _nc_cache


def run(x: np.ndarray, trace: bool = False, tmpdir: str | None = None):
    """Run on 8 NeuronCores; returns (out, BassKernelResults)."""
    x = np.ascontiguousarray(np.asarray(x, dtype=np.float32))
    assert x.shape == (B_FULL, S, C), x.shape
    nc = _get_nc()
    in_maps = [
        {"x": x[i * B : (i + 1) * B]} for i in range(N_CORES)
    ]
    res = run_bass_kernel_spmd(
        nc, in_maps, list(range(N_CORES)), trace=trace, tmpdir=tmpdir
    )
    out = np.concatenate([r["out"] for r in res.results], axis=0)
    return out, res


def kernel(x: np.ndarray) -> np.ndarray:
    out, _ = run(x)
    return out


# revision 8
# speedup vs baseline: 1.6361x; 1.6221x over previous
"""Causal 7-tap sliding-window kernel for Trainium2 (Bass, pure DMA).

out[b, i, j, c] = x[b, i - (6 - j), c], zeros where the index is negative.

Key fact: out[b, i, :, :] (7*128 floats = 3584 B) is the contiguous slice
x[b, i-6 : i+1, :].  So the whole output is produced by DMA alone:

 - shard batch 32 -> 8 cores x 4 batches (pure data parallel)
 - per core, stage x in SBUF as overlapping chunks: chunk (b, c) holds
   x[b, 125c-6 : 125c+125, :] (131 rows, 67072 B) on partition 32b + 2c,
   so every output window lives inside a single partition
 - chunk (b, 0) instead holds [zeros(6 rows); x[b, 0:125]] (zero halo)
 - store: per batch, a 3-dim DMA whose source slides a 3584 B window in
   512 B steps across the chunks and whose destination is one contiguous
   7.17 MB stream of out[b]

HBM traffic per core: 4.3 MB read + 28.7 MB write  (roofline ~92 us).
"""
import sys

import numpy as np

sys.path.insert(0, "/opt/trn_rl_repo")

import concourse.bass as bass
import concourse.mybir as mybir
from concourse.bass_utils import run_bass_kernel_spmd

DT = mybir.dt.float32
N_CORES = 8
B_FULL = 32    # full batch
B = 4          # batches per core
S = 2000       # sequence length
C = 128        # channels
TAPS = 7
NCHUNK = 16    # chunks per batch
ROWS = S // NCHUNK          # 125 output rows per chunk
CROWS = ROWS + TAPS - 1     # 131 rows stored per chunk
RowE = C                    # 128 elems per row (512 B)
CHUNK_E = CROWS * RowE      # 16768 elems per chunk (67072 B)
WIN_E = TAPS * RowE         # 896 elems per output row (3584 B)
HALO_E = (TAPS - 1) * RowE  # 768 elems of zero halo


def build(sim_pad: bool = False, detect_races: bool = True) -> bass.Bass:
    nc = bass.Bass(detect_race_conditions=detect_races)
    x = nc.declare_dram_parameter("x", [B, S, C], DT, isOutput=False)
    out = nc.declare_dram_parameter("out", [B, S, TAPS, C], DT, isOutput=True)

    # chunk (b, c) -> partition 8c + b: every batch's 16 chunks hit all 16
    # SBUF AXI port groups (port = partition // 8), so a single batch-store
    # can pull the full fabric bandwidth.
    def emit_stores(eng, b, store_sem):
        # out[b] is one contiguous 7.17 MB stream; window i reads
        # 3584 B at byte offset 512*(i%125) of partition 8*(i//125) + b.
        # Chunk 0 split from chunks 1-15 (CoreSim init-tracker quirk).
        src_a = bass.AP(
            xs,
            b * CHUNK_E,
            [[CHUNK_E, 1], [RowE, ROWS], [1, WIN_E]],
        )
        dst_a = bass.AP(out, b * S * WIN_E, [[1, ROWS * WIN_E]])
        eng.dma_start(out=dst_a, in_=src_a).then_inc(store_sem, 16)
        src_b = bass.AP(
            xs,
            (8 + b) * CHUNK_E,
            [[8 * CHUNK_E, NCHUNK - 1], [RowE, ROWS], [1, WIN_E]],
        )
        dst_b = bass.AP(
            out,
            (b * S + ROWS) * WIN_E,
            [[1, (S - ROWS) * WIN_E]],
        )
        eng.dma_start(out=dst_b, in_=src_b).then_inc(store_sem, 16)

    with (
        nc.sbuf_tensor([128, CHUNK_E], DT) as xs,
        nc.semaphore("z_sem") as z_sem,
        nc.semaphore("ld0") as ld0,
        nc.semaphore("ld1") as ld1,
        nc.semaphore("ld2") as ld2,
        nc.semaphore("ld3") as ld3,
        nc.semaphore("st_sp") as st_sp,
        nc.semaphore("st_act") as st_act,
        nc.Block() as block,
    ):
        ld = [ld0, ld1, ld2, ld3]

        @block.vector
        def _(vector):
            if sim_pad:
                # CoreSim's uninitialized-memory tracker false-positives on
                # the window reads; pre-touch everything (sim builds only).
                vector.memset(xs[:, :], 0.0)
            # zero halo on every partition (DVE memsets must start at a
            # 32-aligned partition; loads overwrite it on non-halo chunks)
            vector.memset(xs[:, 0:HALO_E], 0.0).then_inc(z_sem, 1)

        @block.scalar
        def _(scalar):
            scalar.wait_ge(z_sem, 1)  # loads overwrite the halo memset
            for b in range(B):
                # c=0: x[b, 0:125, :] -> partition b, after the halo
                dst0 = bass.AP(
                    xs, b * CHUNK_E + HALO_E, [[CHUNK_E, 1], [1, ROWS * RowE]]
                )
                scalar.dma_start(out=dst0, in_=x[b, 0:ROWS, :]).then_inc(ld[b], 16)
                # c=1..15: x[b, 125c-6 : 125c+125, :] -> partition 8c+b
                src = bass.AP(
                    x,
                    b * S * C + (ROWS - (TAPS - 1)) * C,
                    [[ROWS * C, NCHUNK - 1], [1, CHUNK_E]],
                )
                dst = bass.AP(
                    xs, (8 + b) * CHUNK_E, [[8 * CHUNK_E, NCHUNK - 1], [1, CHUNK_E]]
                )
                scalar.dma_start(out=dst, in_=src).then_inc(ld[b], 16)
            for b in (2, 3):
                scalar.wait_ge(ld[b], 32)
                emit_stores(scalar, b, st_act)
            scalar.wait_ge(st_act, 64)

        # stores split across both HWDGE rings (two descriptor generators);
        # each batch-store already covers all 16 ports.
        @block.sync
        def _(sync):
            sync.wait_ge(z_sem, 1)
            for b in (0, 1):
                sync.wait_ge(ld[b], 32)
                emit_stores(sync, b, st_sp)
            sync.wait_ge(st_sp, 64)
            sync.wait_ge(st_act, 64)

    return nc


_nc_cache = None


def _get_nc():
    global _nc_cache
    if _nc_cache is None:
        _nc_cache = build()
    return _nc_cache


def run(x: np.ndarray, trace: bool = False, tmpdir: str | None = None):
    """Run on 8 NeuronCores; returns (out, BassKernelResults)."""
    x = np.ascontiguousarray(np.asarray(x, dtype=np.float32))
    assert x.shape == (B_FULL, S, C), x.shape
    nc = _get_nc()
    in_maps = [
        {"x": x[i * B : (i + 1) * B]} for i in range(N_CORES)
    ]
    res = run_bass_kernel_spmd(
        nc, in_maps, list(range(N_CORES)), trace=trace, tmpdir=tmpdir
    )
    out = np.concatenate([r["out"] for r in res.results], axis=0)
    return out, res


def kernel(x: np.ndarray) -> np.ndarray:
    out, _ = run(x)
    return out
